# revision 5
# baseline (speedup 1.0000x reference)
"""BetaTCVAE loss kernel for 8 TRN2 NeuronCores (Bass/Tile).

Math
----
reference:  out = (BETA-1)*tc + sum(kl)
  lp[i,j,d] = -0.5*((z_i - m_j)^2 * exp(-lv_j) + lv_j + LOG2PI)   (per dim d)
  log_qz_product[i] = sum_d logsumexp_j lp[i,j,d]
  log_qz[i]         = logsumexp_j sum_d lp[i,j,d]
  tc = mean_i(log_qz - log_qz_product)

Decomposition used here (per core, rows i sharded 256/core):
  * log_qz: S'[i,j] = sum_d(-0.5*w*z^2 + w*m*z - 0.5*(w*m^2+lv)) is a pair of
    [256x64]@[64x2048] matmuls plus a rank-1 term -> TensorEngine;
    log_qz[i] = logsumexp_j S'[i,j] - 32*LOG2PI.
  * log_qz_product: A[i,d] = sum_j q*exp(-0.5*w*(z-m)^2). With s=sqrt(w/2)
    the weight q = exp(-0.5*(lv+LOG2PI)) equals s/sqrt(pi), and
    exp(-0.5*w*(z-m)^2) = (sqrt(pi)/2)*DerivErf(s*z - s*m), so
      A_acc[i,d] = sum_j s * DerivErf(s*z - s*m) = 2*A[i,d].
    One ACT instruction per j-column batch (Derivative_Erf), one fused
    scalar_tensor_tensor accumulate per column on DVE/Pool.
  * Partition layout for the hot loop: p = (e,d), e = j-half, d = latent dim;
    free axis = all 256 local i. 1024 packed columns.
  * Final: out = (BETA-1)*(T_sum/B + K0) + KL_sum,
    K0 = -32*LOG2PI + 64*ln2  (host side, exact).
"""

import math
import sys

import numpy as np

if "/opt/trn_rl_repo" not in sys.path:
    sys.path.insert(0, "/opt/trn_rl_repo")

import concourse.bacc as bacc
import concourse.tile as tile
from concourse import mybir
from concourse.bass_utils import run_bass_kernel_spmd
from concourse.masks import make_identity

B, D, M = 2048, 64, 8
BL = B // M          # 256 local rows
NJT = B // 128       # 16 natural j-tiles
NCOL = B // 2        # 1024 packed columns (e-packing: j-halves on partitions)
KB = 8               # j-columns per DerivErf batch
F32 = mybir.dt.float32
BF16 = mybir.dt.bfloat16
LOG_2PI = math.log(2.0 * math.pi)
BETA = 6.0
K0 = -32.0 * LOG_2PI + 64.0 * math.log(2.0)

A = mybir.AluOpType
AF = mybir.ActivationFunctionType
AX = mybir.AxisListType


def _body(tc):
    nc = tc.nc
    kl_ext = nc.dram_tensor("kl", [BL, D], F32, kind="ExternalInput").ap()
    zm_ext = nc.dram_tensor("z_mean", [B, D], F32, kind="ExternalInput").ap()
    zlv_ext = nc.dram_tensor("z_logvar", [B, D], F32, kind="ExternalInput").ap()
    zs_ext = nc.dram_tensor("z_sampled", [BL, D], F32, kind="ExternalInput").ap()
    out_ext = nc.dram_tensor("out", [1, 2], F32, kind="ExternalOutput").ap()

    with (
        tc.tile_pool(name="cst", bufs=1) as cst,
        tc.tile_pool(name="mats", bufs=1) as mats,
        tc.tile_pool(name="ld", bufs=4) as ld,
        tc.tile_pool(name="yb", bufs=3) as yb,
        tc.tile_pool(name="db", bufs=3) as db,
    ):
        ident = cst.tile([128, 128], F32, tag="ident")
        make_identity(nc, ident)
        ones = cst.tile([128, 1], F32, tag="ones")
        nc.vector.memset(ones, 1.0)
        neghalf = cst.tile([128, 128], F32, tag="neghalf")
        nc.gpsimd.memset(neghalf, -0.5)

        # ---- load + transpose z_mean, z_logvar -> M_T/LV_T [64, 2048] ----
        m_t = mats.tile([64, B], F32, tag="m_t")
        lv_t = mats.tile([64, B], F32, tag="lv_t")
        z_t = mats.tile([64, BL], F32, tag="z_t")
        with tc.tile_pool(name="pst", bufs=4, space="PSUM") as pst:
            for t in range(NJT):
                nat = ld.tile([128, D], F32, tag="nat")
                nc.sync.dma_start(out=nat, in_=zm_ext[t * 128:(t + 1) * 128, :])
                ps = pst.tile([64, 128], F32, tag="tp")
                nc.tensor.transpose(ps, nat, ident)
                nc.vector.tensor_copy(out=m_t[0:64, t * 128:(t + 1) * 128], in_=ps)
            for t in range(NJT):
                nat = ld.tile([128, D], F32, tag="nat")
                nc.sync.dma_start(out=nat, in_=zlv_ext[t * 128:(t + 1) * 128, :])
                ps = pst.tile([64, 128], F32, tag="tp")
                nc.tensor.transpose(ps, nat, ident)
                nc.vector.tensor_copy(out=lv_t[0:64, t * 128:(t + 1) * 128], in_=ps)
            for t in range(2):
                nat = ld.tile([128, D], F32, tag="nat")
                nc.sync.dma_start(out=nat, in_=zs_ext[t * 128:(t + 1) * 128, :])
                ps = pst.tile([64, 128], F32, tag="tp")
                nc.tensor.transpose(ps, nat, ident)
                nc.vector.tensor_copy(out=z_t[0:64, t * 128:(t + 1) * 128], in_=ps)

        # ---- kl partial sum ----
        ks2 = mats.tile([128, 2], F32, tag="ks2")
        for t in range(2):
            klt = ld.tile([128, D], F32, tag="klt", bufs=2)
            nc.sync.dma_start(out=klt, in_=kl_ext[t * 128:(t + 1) * 128, :])
            nc.vector.tensor_reduce(out=ks2[:, t:t + 1], in_=klt, axis=AX.X, op=A.add)
        kss = mats.tile([128, 1], F32, tag="kss")
        nc.vector.tensor_reduce(out=kss, in_=ks2, axis=AX.X, op=A.add)

        # ---- prep params (T-layout, [64, 2048]) ----
        s_t = mats.tile([64, B], F32, tag="s_t")
        #  s = exp(-lv/2)/sqrt(2) = sqrt(w/2)
        bias_l2 = cst.tile([128, 1], F32, tag="bias_l2")
        nc.gpsimd.memset(bias_l2, -0.5 * math.log(2.0))
        nc.scalar.activation(out=s_t[0:64, :], in_=lv_t[0:64, :], func=AF.Exp,
                             bias=bias_l2[0:64, :], scale=-0.5)
        nsm_t = mats.tile([64, B], F32, tag="nsm_t")
        nc.vector.scalar_tensor_tensor(out=nsm_t[0:64, :], in0=m_t[0:64, :],
                                       scalar=-1.0, in1=s_t[0:64, :],
                                       op0=A.mult, op1=A.mult)
        w_t = mats.tile([64, B], F32, tag="w_t")
        nc.vector.scalar_tensor_tensor(out=w_t[0:64, :], in0=s_t[0:64, :],
                                       scalar=2.0, in1=s_t[0:64, :],
                                       op0=A.mult, op1=A.mult)
        wm_t = mats.tile([64, B], F32, tag="wm_t")
        nc.vector.scalar_tensor_tensor(out=wm_t[0:64, :], in0=nsm_t[0:64, :],
                                       scalar=-2.0, in1=s_t[0:64, :],
                                       op0=A.mult, op1=A.mult)
        t3 = mats.tile([64, B], F32, tag="t3")
        nc.gpsimd.tensor_mul(out=t3[0:64, :], in0=wm_t[0:64, :], in1=m_t[0:64, :])
        nc.gpsimd.tensor_add(out=t3[0:64, :], in0=t3[0:64, :], in1=lv_t[0:64, :])

        z2n_t = mats.tile([64, BL], F32, tag="z2n_t")
        nc.scalar.activation(out=z2n_t[0:64, :], in_=z_t[0:64, :], func=AF.Square,
                             bias=0.0, scale=1.0)
        nc.vector.tensor_scalar(out=z2n_t[0:64, :], in0=z2n_t[0:64, :],
                                scalar1=-0.5, scalar2=None, op0=A.mult)

        # ---- packed tiles for the hot loop ----
        s_pk = mats.tile([128, NCOL], F32, tag="s_pk")
        nsm_pk = mats.tile([128, NCOL], F32, tag="nsm_pk")
        nc.sync.dma_start(out=s_pk[0:64, :], in_=s_t[0:64, 0:NCOL])
        nc.sync.dma_start(out=s_pk[64:128, :], in_=s_t[0:64, NCOL:B])
        nc.sync.dma_start(out=nsm_pk[0:64, :], in_=nsm_t[0:64, 0:NCOL])
        nc.sync.dma_start(out=nsm_pk[64:128, :], in_=nsm_t[0:64, NCOL:B])
        zrep = mats.tile([128, BL], F32, tag="zrep")
        nc.sync.dma_start(out=zrep[0:64, :], in_=z_t[0:64, :])
        nc.sync.dma_start(out=zrep[64:128, :], in_=z_t[0:64, :])
        zrep_bf = mats.tile([128, BL], BF16, tag="zrep_bf")
        nc.vector.tensor_copy(out=zrep_bf, in_=zrep)

        # ---- accumulator (f32, folded by Pool once per block) ----
        a_tot = mats.tile([128, BL], F32, tag="a_tot")
        nc.vector.memset(a_tot, 0.0)

        # ---- HOT LOOP ----
        # Per block of KB columns: y-ops (Pool/DVE split), one DerivErf batch
        # on ACT, bf16 weighted accumulate into a block partial on DVE
        # (2x mode), one f32 fold on Pool.
        nblk = NCOL // KB
        with tc.tile_pool(name="pb", bufs=3) as pb:
            for blk in range(nblk):
                ybt = yb.tile([128, KB, BL], BF16, tag="yblk")
                for k in range(KB):
                    jc = blk * KB + k
                    eng = nc.vector if (jc % 4 == 0) else nc.gpsimd
                    eng.tensor_scalar(out=ybt[:, k, :], in0=zrep_bf,
                                      scalar1=s_pk[:, jc:jc + 1],
                                      scalar2=nsm_pk[:, jc:jc + 1],
                                      op0=A.mult, op1=A.add)
                dbt = db.tile([128, KB, BL], BF16, tag="dblk")
                nc.scalar.activation(out=dbt, in_=ybt, func=AF.Derivative_Erf,
                                     bias=0.0, scale=1.0)
                part = pb.tile([128, BL], BF16, tag="part")
                jc0 = blk * KB
                nc.vector.tensor_scalar(out=part, in0=dbt[:, 0, :],
                                        scalar1=s_pk[:, jc0:jc0 + 1],
                                        scalar2=None, op0=A.mult)
                for k in range(1, KB):
                    jc = blk * KB + k
                    nc.vector.scalar_tensor_tensor(out=part, in0=dbt[:, k, :],
                                                   scalar=s_pk[:, jc:jc + 1],
                                                   in1=part, op0=A.mult,
                                                   op1=A.add)
                nc.gpsimd.tensor_add(out=a_tot, in0=a_tot, in1=part)

        # ---- A epilogue: fold halves, log, partition-reduce ----
        a_hi = mats.tile([64, BL], F32, tag="a_hi")
        nc.sync.dma_start(out=a_hi[0:64, :], in_=a_tot[64:128, :])
        a_fold = mats.tile([64, BL], F32, tag="a_fold")
        nc.vector.tensor_add(out=a_fold[0:64, :], in0=a_tot[0:64, :],
                             in1=a_hi[0:64, :])
        ln_a = mats.tile([64, BL], F32, tag="ln_a")
        nc.scalar.activation(out=ln_a[0:64, :], in_=a_fold[0:64, :], func=AF.Ln,
                             bias=0.0, scale=1.0)

        # ---- S' matmuls + logsumexp epilogue ----
        contrib = []
        with (
            tc.tile_pool(name="psp", bufs=1, space="PSUM") as psp,
            tc.tile_pool(name="psm", bufs=2, space="PSUM") as psm,
            tc.tile_pool(name="scr", bufs=2) as scr,
        ):
            for it in range(2):
                isl = slice(it * 128, (it + 1) * 128)
                sps = []
                for jb in range(4):
                    jsl = slice(jb * 512, (jb + 1) * 512)
                    sp = psp.tile([128, 512], F32, tag=f"sp{jb}")
                    nc.tensor.matmul(sp, lhsT=z2n_t[0:64, isl], rhs=w_t[0:64, jsl],
                                     start=True, stop=False)
                    nc.tensor.matmul(sp, lhsT=z_t[0:64, isl], rhs=wm_t[0:64, jsl],
                                     start=False, stop=False)
                    nc.tensor.matmul(sp, lhsT=neghalf[0:64, :], rhs=t3[0:64, jsl],
                                     start=False, stop=True)
                    sps.append(sp)
                mx4 = mats.tile([128, 4], F32, tag="mx4", bufs=2)
                for jb in range(4):
                    nc.vector.tensor_reduce(out=mx4[:, jb:jb + 1], in_=sps[jb],
                                            axis=AX.X, op=A.max)
                nmx = mats.tile([128, 1], F32, tag="nmx", bufs=2)
                nc.vector.tensor_reduce(out=nmx, in_=mx4, axis=AX.X, op=A.max,
                                        negate=True)
                es4 = mats.tile([128, 4], F32, tag="es4", bufs=2)
                for jb in range(4):
                    sc = scr.tile([128, 512], F32, tag="sc")
                    nc.scalar.activation(out=sc, in_=sps[jb], func=AF.Exp,
                                         bias=nmx, scale=1.0,
                                         accum_out=es4[:, jb:jb + 1])
                esum = mats.tile([128, 1], F32, tag="esum", bufs=2)
                nc.vector.tensor_reduce(out=esum, in_=es4, axis=AX.X, op=A.add)
                lqz = mats.tile([128, 1], F32, tag="lqz", bufs=2)
                nc.scalar.activation(out=lqz, in_=esum, func=AF.Ln,
                                     bias=0.0, scale=1.0)
                # lqz - P  (P via ones-matmul over d), both [128,1]
                pps = psm.tile([128, 1], F32, tag="pp")
                nc.tensor.matmul(pps, lhsT=ln_a[0:64, isl], rhs=ones[0:64, :],
                                 start=True, stop=True)
                ctr = mats.tile([128, 1], F32, tag="ctr", bufs=2)
                # ctr = (lqz + (-1)*mx4_max...) careful: lqz currently ln(esum);
                # full log_qz = lqz + mx ; contrib = lqz + mx - P
                mx = mats.tile([128, 1], F32, tag="mx", bufs=2)
                nc.vector.tensor_scalar(out=mx, in0=nmx, scalar1=-1.0,
                                        scalar2=None, op0=A.mult)
                nc.vector.tensor_add(out=lqz, in0=lqz, in1=mx)
                nc.vector.tensor_sub(out=ctr, in0=lqz, in1=pps)
                contrib.append(ctr)

            # ---- final scalars ----
            fps = psm.tile([1, 2], F32, tag="fps")
            nc.tensor.matmul(fps[0:1, 0:1], lhsT=contrib[0], rhs=ones,
                             start=True, stop=False)
            nc.tensor.matmul(fps[0:1, 0:1], lhsT=contrib[1], rhs=ones,
                             start=False, stop=True)
            nc.tensor.matmul(fps[0:1, 1:2], lhsT=kss, rhs=ones,
                             start=True, stop=True)
            out_sb = mats.tile([1, 2], F32, tag="out_sb")
            nc.vector.tensor_copy(out=out_sb[0:1, :], in_=fps[0:1, :])
            nc.sync.dma_start(out=out_ext, in_=out_sb[0:1, :])


_NC_CACHE = {}


def _get_nc():
    if "nc" not in _NC_CACHE:
        nc = bacc.Bacc("TRN2", target_bir_lowering=False, debug=False,
                       num_devices=M)
        with tile.TileContext(nc) as tc:
            _body(tc)
        nc.compile()
        _NC_CACHE["nc"] = nc
    return _NC_CACHE["nc"]


def kernel(kl, z_mean, z_logvar, z_sampled, _trace=False, _tmpdir=None):
    kl = np.ascontiguousarray(kl, dtype=np.float32)
    z_mean = np.ascontiguousarray(z_mean, dtype=np.float32)
    z_logvar = np.ascontiguousarray(z_logvar, dtype=np.float32)
    z_sampled = np.ascontiguousarray(z_sampled, dtype=np.float32)
    nc = _get_nc()
    in_maps = []
    for c in range(M):
        sl = slice(c * BL, (c + 1) * BL)
        in_maps.append({
            "kl": np.ascontiguousarray(kl[sl]),
            "z_mean": z_mean,
            "z_logvar": z_logvar,
            "z_sampled": np.ascontiguousarray(z_sampled[sl]),
        })
    res = run_bass_kernel_spmd(nc, in_maps, list(range(M)), trace=_trace,
                               tmpdir=_tmpdir)
    t_sum = 0.0
    kl_sum = 0.0
    for c in range(M):
        o = res.results[c]["out"]
        t_sum += float(o[0, 0])
        kl_sum += float(o[0, 1])
    val = (BETA - 1.0) * (t_sum / B + K0) + kl_sum
    out = np.float32(val)
    if _trace:
        return out, res
    return out
